# revision 1
# baseline (speedup 1.0000x reference)
"""AdaptiveSparsityAttention TRN2 kernel (8 NeuronCores, SPMD data-parallel).

Problem (B=2, S=1024, D=512, H=2 heads, dh=256, hidden=128):
  q,k,v = x@Wq, x@Wk, x@Wv (split 2 heads); scores = q@k^T/16
  a_i = q_mean@W1[:dh]+b1, c_j = k_mean@W1[dh:]
  z[i,j] = W2 . relu(a_i + c_j)          (sigmoid(z+b2)>0.5  <=>  z > -b2)
  attn = softmax(mask(scores));  out = (attn@v)@Wo + bo

Sharding: 8 cores = 2 batches x 4 query-chunks of 256 rows. Each core
computes its output chunk fully locally (K/V/k_mean recomputed per core
from its batch's x - cheap), no collectives.

Key structure per core (measured ~214us on silicon, rel err 1.4e-3 vs ref):
  - a/c computed EXACTLY (fp32) via host-folded Mq = Wq_mean@W1[:dh],
    Mk = Wk_mean@W1[dh:]. The mask-decision margins are tiny (z std 0.022,
    31% of pairs within 1e-2 of threshold): bf16/tf32-level error in z
    flips enough mask bits to fail; z MUST be fp32-exact.
  - T_i = relu(C^T + a_i) as [128h x 1024j] tiles, split DVE/ACT 9:7
    (DVE tensor_scalar (max,add) 740ns; ACT Relu-with-bias 1049ns;
    GPSIMD tensor_scalar is 15us software - never use)
  - z rows assembled in PSUM via delta-trick: accumulating fp32 matmuls
    with stationary w*e_i (sliding 32-col slice of a host-built buffer),
    4-way col-tiled (tile_position=(0,32g)); groups MUST cycle per step -
    consecutive same-group fp32 matmuls serialize at 2cyc/col.
    Pace: ~215-280ns/instr (fp32 = 2 ldw + 2 passes, ldw-chain-bound).
  - mask applied as multiply on exp(scores); rowmax subtraction dropped
    (scores/16 bounded, exp can't overflow; mathematically identical),
    fully-masked rows get reference's uniform 1/1024 via ind=[s==0] trick
  - scores/AV/projections float32r (1 cyc/row, producers must round);
    E(ti=0) attention interleaved between the two z blocks
Engine budget at 215us: PE ~192us busy (z-stream 143), DVE ~145, ACT ~145.
"""

import sys

if "/opt/trn_rl_repo" not in sys.path:
    sys.path.insert(0, "/opt/trn_rl_repo")

import numpy as np

import concourse.bass as bass  # noqa: F401
import concourse.tile as tile
from concourse import bacc, mybir
from concourse.bass_utils import run_bass_kernel_spmd
from concourse.masks import make_identity

F32 = mybir.dt.float32
F32R = mybir.dt.float32r
AL = mybir.AluOpType
AF = mybir.ActivationFunctionType

B, S, D = 2, 1024, 512
DH = D // 2          # 256 per-head dim
HID = 128            # predictor hidden
NCHUNK = S // 4      # 256 query rows per core
P = 128

# knobs (test.py may override before first kernel() call)
CONFIG = {
    "coltile": True,       # 4-way col-tiled z matmuls
    "trace": False,
    "tmpdir": None,
    # T-producer rotation per 16 rows: 'v'=DVE, 's'=ACT, 'g'=GPSIMD
    # (GPSIMD tensor_scalar is ~15us software emulation - never use it)
    "tpat": ["v", "s", "v", "s", "v", "v", "s", "v", "s", "v", "v", "s", "v", "s", "v", "s"],
    "t_bufs": 11,
    "f32r_dma": True,   # DMA weights directly as float32r (skip staging+convert)
    "zbank_split": False,  # split z PSUM banks by col-group (measured: no effect)
}

_STATE = {}


def _emit(tc, nc, t):
    sl512 = [slice(0, 512), slice(512, 1024)]
    f32r_dma = CONFIG["f32r_dma"]
    zsplit = CONFIG["zbank_split"] and CONFIG["coltile"]

    with tc.tile_pool(name="big", bufs=1) as big:
        # ---- small DMAs ----
        mq_s = big.tile([P, 4, HID], F32, name="mq_s")
        mk_s = big.tile([P, 4, HID], F32, name="mk_s")
        for t_ in range(4):
            nc.sync.dma_start(mq_s[:, t_, :], t["mq"][128 * t_ : 128 * (t_ + 1), :])
            nc.sync.dma_start(mk_s[:, t_, :], t["mk"][128 * t_ : 128 * (t_ + 1), :])
        b1_s = big.tile([P, 1], F32, name="b1_s")
        nc.sync.dma_start(b1_s[:], t["b1c"])
        thr_s = big.tile([P, 1], F32, name="thr_s")
        nc.sync.dma_start(thr_s[:], t["thr"])
        wsel_s = big.tile([P, 64], F32, name="wsel_s")
        nc.sync.dma_start(wsel_s[:], t["wsel32"])
        if not CONFIG["coltile"]:
            wself_s = big.tile([P, 256], F32, name="wself_s")
            nc.sync.dma_start(wself_s[:], t["wself"])
        bo_s = big.tile([1, D], F32, name="bo_s")
        nc.sync.dma_start(bo_s[:], t["bo2"])
        one_s = big.tile([1, P], F32, name="one_s")
        nc.sync.dma_start(one_s[:], t["one1"])

        # ---- persistent residents ----
        at_s = big.tile([P, NCHUNK], F32, name="at_s")    # a^T + b1, [h, i]
        nat_s = big.tile([P, NCHUNK], F32, name="nat_s")  # -(a^T + b1)
        ct_s = big.tile([P, S], F32, name="ct_s")          # c^T, [h, j]
        xqr_s = big.tile([P, 4, NCHUNK], F32R, name="xqr_s")
        xtr_s = big.tile([P, 4, S], F32R, name="xtr_s")
        wdt = F32R if f32r_dma else F32
        wqr_s = big.tile([P, 4, D], wdt, name="wqr_s")
        wkr_s = big.tile([P, 4, D], wdt, name="wkr_s")
        wvr_s = big.tile([P, 4, D], wdt, name="wvr_s")
        wor_s = big.tile([P, 4, D], wdt, name="wor_s")
        qt_s = big.tile([P, 4, NCHUNK], F32R, name="qt_s")  # q^T/16 [dout, i]
        kt_s = big.tile([P, 4, S], F32R, name="kt_s")       # k^T [dout, j]
        v_s = big.tile([P, 8, D], F32R, name="v_s")         # v [j(8 tiles), d]
        ident = big.tile([P, P], F32, name="ident")
        make_identity(nc, ident[:])
        bor_s = big.tile([1, D], F32R, name="bor_s")
        oner_s = big.tile([1, P], F32R, name="oner_s")
        otr_s = big.tile([P, 4, NCHUNK], F32R, name="otr_s")  # out^T [d, i]
        mask_s = [big.tile([P, S], F32, name=f"mask{b_}") for b_ in range(2)]

        with (
            tc.tile_pool(name="pjp", bufs=1, space="PSUM") as pjp,
            tc.tile_pool(name="zps", bufs=1, space="PSUM") as zpsp,
        ):
            # ---------------- stage A/B (transient f32 staging) ----
            with tc.tile_pool(name="stageA", bufs=1) as sa:
                # x DMAs first (critical path for stage B/D); weights after
                xq_s = sa.tile([P, 4, NCHUNK], F32, name="xq_s")
                xt_s = sa.tile([P, 4, S], F32, name="xt_s")
                for t_ in range(4):
                    nc.sync.dma_start(xq_s[:, t_, :], t["xqT"][128 * t_ : 128 * (t_ + 1), :])
                for t_ in range(4):
                    nc.sync.dma_start(xt_s[:, t_, :], t["xT"][128 * t_ : 128 * (t_ + 1), :])
                if f32r_dma:
                    for t_ in range(4):
                        nc.sync.dma_start(wqr_s[:, t_, :], t["wq"][128 * t_ : 128 * (t_ + 1), :])
                    for t_ in range(4):
                        nc.sync.dma_start(wkr_s[:, t_, :], t["wk"][128 * t_ : 128 * (t_ + 1), :])
                    for t_ in range(4):
                        nc.sync.dma_start(wvr_s[:, t_, :], t["wv"][128 * t_ : 128 * (t_ + 1), :])
                    for t_ in range(4):
                        nc.sync.dma_start(wor_s[:, t_, :], t["wo"][128 * t_ : 128 * (t_ + 1), :])

                # exact a/c (fp32 matmuls)
                at_ps = pjp.tile([P, NCHUNK], F32, tag="vps", bufs=2, name="at_ps")
                for dt_ in range(4):
                    nc.tensor.matmul(
                        at_ps[:], mq_s[:, dt_, :], xq_s[:, dt_, :],
                        start=(dt_ == 0), stop=(dt_ == 3),
                    )
                nc.vector.tensor_scalar(at_s[:], at_ps[:], b1_s[:], None, AL.add)
                nc.vector.tensor_scalar(nat_s[:], at_s[:], -1.0, None, AL.mult)

                for jc in range(2):
                    ct_ps = pjp.tile([P, 512], F32, tag="vps", bufs=2, name="ct_ps")
                    for dt_ in range(4):
                        nc.tensor.matmul(
                            ct_ps[:], mk_s[:, dt_, :], xt_s[:, dt_, sl512[jc]],
                            start=(dt_ == 0), stop=(dt_ == 3),
                        )
                    nc.scalar.copy(ct_s[:, sl512[jc]], ct_ps[:])


                if not f32r_dma:
                    for wi, (nm, dst) in enumerate(
                        [("wq", wqr_s), ("wk", wkr_s), ("wv", wvr_s), ("wo", wor_s)]
                    ):
                        ws = sa.tile([P, 4, D], F32, tag="wstage", bufs=2, name=f"ws_{nm}")
                        for t_ in range(4):
                            nc.sync.dma_start(
                                ws[:, t_, :], t[nm][128 * t_ : 128 * (t_ + 1), :]
                            )
                        if wi % 2 == 0:
                            nc.vector.tensor_copy(dst[:], ws[:])
                        else:
                            nc.scalar.copy(dst[:], ws[:])

                # f32r conversions of x: emitted here so their FIFO slots don't
                # block T production, but they only gate stage C / QT below
                nc.vector.tensor_copy(xqr_s[:], xq_s[:])
                nc.vector.tensor_copy(xtr_s[:], xt_s[:])
            # ---------------- stage D + C + E + F (interleaved) ----------------
            with (
                tc.tile_pool(name="Tp", bufs=CONFIG["t_bufs"]) as Tp,
                tc.tile_pool(name="work", bufs=2) as work,
            ):
                def emit_zblock(blk):
                    # z accumulation for 128 query rows; col groups MUST cycle
                    # (g=step%4): serial same-group fp32 mms run at 2cyc/col (~426ns),
                    # cycling pipelines the 2-pass across groups (~135ns effective)
                    zp = zpsp.tile([P, S], F32, tag="z", bufs=2, name=f"zp{blk}")
                    for step in range(128):
                        if CONFIG["coltile"]:
                            k, g = step // 4, step % 4
                            i = 32 * g + k
                        else:
                            k, g, i = step, 0, step
                        ii = blk * 128 + i
                        T = Tp.tile([P, S], F32, tag="T", name=f"T{ii}")
                        eng = CONFIG["tpat"][ii % 16]
                        if eng == "v":
                            # relu(ct + a) as max(ct, -a) + a  ((add,max) hits a
                            # slow DVE path in context; (max,add) measured 723ns)
                            nc.vector.tensor_scalar(
                                T[:], ct_s[:], nat_s[:, ii : ii + 1],
                                at_s[:, ii : ii + 1], AL.max, AL.add,
                            )
                        elif eng == "s":
                            nc.scalar.activation(
                                T[:], ct_s[:], AF.Relu, bias=at_s[:, ii : ii + 1]
                            )
                        else:
                            nc.gpsimd.tensor_scalar(
                                T[:], ct_s[:], at_s[:, ii : ii + 1], 0.0, AL.add, AL.max
                            )
                        for jc in range(2):
                            if CONFIG["coltile"]:
                                nc.tensor.matmul(
                                    zp[32 * g : 32 * g + 32, sl512[jc]],
                                    wsel_s[:, 32 - k : 64 - k],
                                    T[:, sl512[jc]],
                                    start=(k == 0), stop=(k == 31),
                                    tile_position=(0, 32 * g),
                                    skip_group_check=True,
                                )
                            else:
                                nc.tensor.matmul(
                                    zp[:, sl512[jc]],
                                    wself_s[:, 128 - i : 256 - i],
                                    T[:, sl512[jc]],
                                    start=(i == 0), stop=(i == 127),
                                )
                    m01 = mask_s[blk]
                    for jc in range(2):
                        nc.vector.tensor_scalar(
                            m01[:, sl512[jc]], zp[:, sl512[jc]],
                            thr_s[:], None, AL.is_gt,
                        )

                # per-head attnT accumulators (written in ti halves)
                att_sb = [
                    [
                        work.tile([P, NCHUNK], F32R, tag="attnT", bufs=16,
                                  name=f"a_sb{h}_{jt}")
                        for jt in range(8)
                    ]
                    for h in range(2)
                ]

                def emit_attn_half(h, ti):
                    # softmax(masked scores) for rows [128*ti, 128*ti+128) of head h,
                    # transposed into att_sb[h][jt][:, 128*ti:...]
                    sc_ps = zpsp.tile([P, S], F32, tag="z", bufs=2, name="sc_ps")
                    for jc in range(2):
                        for dt_ in range(2):
                            nc.tensor.matmul(
                                sc_ps[:, sl512[jc]],
                                qt_s[:, 2 * h + dt_, 128 * ti : 128 * (ti + 1)],
                                kt_s[:, 2 * h + dt_, sl512[jc]],
                                start=(dt_ == 0), stop=(dt_ == 1),
                            )
                    # scores/16 are bounded (|sc| < ~7) so exp never overflows;
                    # skipping the rowmax subtraction is mathematically identical
                    e = work.tile([P, S], F32, tag="e", bufs=1, name="e")
                    nc.scalar.activation(e[:], sc_ps[:], AF.Exp)
                    em = work.tile([P, S], F32, tag="em", name="em")
                    ssum = work.tile([P, 1], F32, tag="ssum", name="ssum")
                    nc.vector.scalar_tensor_tensor(
                        em[:], e[:], 0.0, mask_s[ti][:], AL.add, AL.mult,
                        accum_out=ssum[:],
                    )
                    # fully-masked rows: reference = uniform 1/1024.
                    # ind = [s==0]; attn = (em + ind) / (s + 1024*ind)
                    ind = work.tile([P, 1], F32, tag="ind", name="ind")
                    nc.vector.tensor_scalar(ind[:], ssum[:], 0.0, None, AL.is_equal)
                    s2 = work.tile([P, 1], F32, tag="s2", name="s2")
                    nc.vector.tensor_scalar(s2[:], ind[:], 1024.0, ssum[:], AL.mult, AL.add)
                    rinv = work.tile([P, 1], F32, tag="rinv", name="rinv")
                    nc.vector.reciprocal(rinv[:], s2[:])
                    nc.vector.tensor_scalar(em[:], em[:], ind[:], rinv[:], AL.add, AL.mult)
                    for jt in range(8):
                        tp_ps = pjp.tile([P, P], F32, tag="tp", bufs=2, name="tp_ps")
                        nc.tensor.transpose(
                            tp_ps[:], em[:, 128 * jt : 128 * (jt + 1)], ident[:]
                        )
                        dst = att_sb[h][jt][:, 128 * ti : 128 * (ti + 1)]
                        if jt % 2 == 0:
                            nc.vector.tensor_copy(dst, tp_ps[:])
                        else:
                            nc.scalar.copy(dst, tp_ps[:])

                def emit_av(h):
                    for dt_ in range(2):
                        ot_ps = pjp.tile([P, NCHUNK], F32, tag="vps", bufs=2, name="ot_ps")
                        for jt in range(8):
                            nc.tensor.matmul(
                                ot_ps[:],
                                v_s[:, jt, 256 * h + 128 * dt_ : 256 * h + 128 * (dt_ + 1)],
                                att_sb[h][jt][:],
                                start=(jt == 0), stop=(jt == 7),
                            )
                        if dt_ == 0:
                            nc.vector.tensor_copy(otr_s[:, 2 * h + dt_, :], ot_ps[:])
                        else:
                            nc.scalar.copy(otr_s[:, 2 * h + dt_, :], ot_ps[:])

                # ---- emission: blk0 -> C -> E(ti=0) -> blk1 -> E(ti=1)+AV -> F
                emit_zblock(0)

                # Q^T (f32r) + 1/16 scale folded into the PSUM->SBUF copy.
                # Emitted after blk0 so the ACT-FIFO isn't blocked waiting on
                # the wq DMA ahead of T production (costs ~15us of prefix).
                for dout in range(4):
                    qt_ps = pjp.tile([P, NCHUNK], F32, tag="vps", bufs=2, name="qt_ps")
                    for dt_ in range(4):
                        nc.tensor.matmul(
                            qt_ps[:], wqr_s[:, dt_, 128 * dout : 128 * (dout + 1)],
                            xqr_s[:, dt_, :], start=(dt_ == 0), stop=(dt_ == 3),
                        )
                    nc.scalar.mul(qt_s[:, dout, :], qt_ps[:], 1.0 / 16.0)

                # stage C: K/V projections (f32r), fill PE gaps in blk0
                for dout in range(4):
                    for jc in range(2):
                        kt_ps = pjp.tile([P, 512], F32, tag="vps", bufs=2, name="kt_ps")
                        for dt_ in range(4):
                            nc.tensor.matmul(
                                kt_ps[:],
                                wkr_s[:, dt_, 128 * dout : 128 * (dout + 1)],
                                xtr_s[:, dt_, sl512[jc]],
                                start=(dt_ == 0), stop=(dt_ == 3),
                            )
                        if (dout + jc) % 2 == 0:
                            nc.vector.tensor_copy(kt_s[:, dout, sl512[jc]], kt_ps[:])
                        else:
                            nc.scalar.copy(kt_s[:, dout, sl512[jc]], kt_ps[:])

                for jt in range(8):
                    v_ps = pjp.tile([P, D], F32, tag="vps", bufs=2, name="v_ps")
                    for dt_ in range(4):
                        nc.tensor.matmul(
                            v_ps[:], xtr_s[:, dt_, 128 * jt : 128 * (jt + 1)],
                            wvr_s[:, dt_, :], start=(dt_ == 0), stop=(dt_ == 3),
                        )
                    if jt % 2 == 0:
                        nc.vector.tensor_copy(v_s[:, jt, :], v_ps[:])
                    else:
                        nc.scalar.copy(v_s[:, jt, :], v_ps[:])

                emit_attn_half(0, 0)
                emit_attn_half(1, 0)
                emit_zblock(1)
                emit_attn_half(0, 1)
                emit_av(0)
                emit_attn_half(1, 1)
                emit_av(1)

                # ---- stage F: output projection ----
                nc.vector.tensor_copy(bor_s[:], bo_s[:])
                nc.vector.tensor_copy(oner_s[:], one_s[:])
                for ti in range(2):
                    o_ps = pjp.tile([P, D], F32, tag="vps", bufs=2, name="o_ps")
                    nc.tensor.matmul(o_ps[:], oner_s[:], bor_s[:], start=True, stop=False)
                    for dt_ in range(4):
                        nc.tensor.matmul(
                            o_ps[:], otr_s[:, dt_, 128 * ti : 128 * (ti + 1)],
                            wor_s[:, dt_, :], start=False, stop=(dt_ == 3),
                        )
                    o_sb = work.tile([P, D], F32, tag="osb", bufs=1, name="o_sb")
                    nc.vector.tensor_copy(o_sb[:], o_ps[:])
                    nc.sync.dma_start(t["out"][128 * ti : 128 * (ti + 1), :], o_sb[:])



def _build():
    if "nc" in _STATE:
        return _STATE["nc"]
    nc = bacc.Bacc(
        "TRN2", target_bir_lowering=False, debug=False, enable_asserts=True,
        num_devices=8,
    )
    t = {}
    t["xT"] = nc.dram_tensor("xT", [D, S], F32, kind="ExternalInput").ap()
    t["xqT"] = nc.dram_tensor("xqT", [D, NCHUNK], F32, kind="ExternalInput").ap()
    WDT = F32R if CONFIG["f32r_dma"] else F32
    t["wq"] = nc.dram_tensor("wq", [D, D], WDT, kind="ExternalInput").ap()
    t["wk"] = nc.dram_tensor("wk", [D, D], WDT, kind="ExternalInput").ap()
    t["wv"] = nc.dram_tensor("wv", [D, D], WDT, kind="ExternalInput").ap()
    t["wo"] = nc.dram_tensor("wo", [D, D], WDT, kind="ExternalInput").ap()
    t["mq"] = nc.dram_tensor("mq", [D, HID], F32, kind="ExternalInput").ap()
    t["mk"] = nc.dram_tensor("mk", [D, HID], F32, kind="ExternalInput").ap()
    t["b1c"] = nc.dram_tensor("b1c", [P, 1], F32, kind="ExternalInput").ap()
    t["thr"] = nc.dram_tensor("thr", [P, 1], F32, kind="ExternalInput").ap()
    t["wsel32"] = nc.dram_tensor("wsel32", [P, 64], F32, kind="ExternalInput").ap()
    if not CONFIG["coltile"]:
        t["wself"] = nc.dram_tensor("wself", [P, 256], F32, kind="ExternalInput").ap()
    t["bo2"] = nc.dram_tensor("bo2", [1, D], F32, kind="ExternalInput").ap()
    t["one1"] = nc.dram_tensor("one1", [1, P], F32, kind="ExternalInput").ap()
    t["out"] = nc.dram_tensor("out", [NCHUNK, D], F32, kind="ExternalOutput").ap()

    with tile.TileContext(nc) as tc:
        _emit(tc, nc, t)
    nc.compile()
    _STATE["nc"] = nc
    return nc


def _prep_in_maps(inputs):
    x = np.ascontiguousarray(np.asarray(inputs["x"], np.float32))
    Wq = np.asarray(inputs["Wq"], np.float32)
    Wk = np.asarray(inputs["Wk"], np.float32)
    Wv = np.asarray(inputs["Wv"], np.float32)
    Wo = np.asarray(inputs["Wo"], np.float32)
    bo = np.asarray(inputs["bo"], np.float32)
    W1 = np.asarray(inputs["W1"], np.float64)
    b1 = np.asarray(inputs["b1"], np.float32)
    W2 = np.asarray(inputs["W2"], np.float32)
    b2 = np.asarray(inputs["b2"], np.float32)

    wq_m = 0.5 * (Wq[:, :DH].astype(np.float64) + Wq[:, DH:].astype(np.float64))
    wk_m = 0.5 * (Wk[:, :DH].astype(np.float64) + Wk[:, DH:].astype(np.float64))
    Mq = np.ascontiguousarray((wq_m @ W1[:DH]).astype(np.float32))
    Mk = np.ascontiguousarray((wk_m @ W1[DH:]).astype(np.float32))

    wsel32 = np.zeros((P, 64), np.float32)
    wsel32[:, 32] = W2[:, 0]
    b1c = np.ascontiguousarray(b1.reshape(P, 1))
    thr = np.full((P, 1), -float(b2[0]), np.float32)
    bo2 = np.ascontiguousarray(bo.reshape(1, D))
    one1 = np.ones((1, P), np.float32)

    shared = dict(
        wq=Wq, wk=Wk, wv=Wv, wo=Wo, mq=Mq, mk=Mk, b1c=b1c, thr=thr,
        wsel32=wsel32, bo2=bo2, one1=one1,
    )
    if not CONFIG["coltile"]:
        wself = np.zeros((P, 256), np.float32)
        wself[:, 128] = W2[:, 0]
        shared["wself"] = wself
    in_maps = []
    xT = [np.ascontiguousarray(x[b].T) for b in range(B)]
    for c in range(8):
        b, i0 = c // 4, (c % 4) * NCHUNK
        m = dict(shared)
        m["xT"] = xT[b]
        m["xqT"] = np.ascontiguousarray(x[b, i0 : i0 + NCHUNK].T)
        in_maps.append(m)
    return in_maps


def kernel(**inputs):
    nc = _build()
    in_maps = _prep_in_maps(inputs)
    res = run_bass_kernel_spmd(
        nc, in_maps, core_ids=list(range(8)),
        trace=CONFIG["trace"], tmpdir=CONFIG["tmpdir"],
    )
    _STATE["last_result"] = res
    out = np.empty((B, S, D), np.float32)
    for c in range(8):
        b, i0 = c // 4, (c % 4) * NCHUNK
        out[b, i0 : i0 + NCHUNK] = res.results[c]["out"]
    return out

